# revision 29
# baseline (speedup 1.0000x reference)
"""CPCNet forward on 8 Trainium2 NeuronCores (Bass/Tile).  v7

Data-parallel over batch: each core does 16 of 128 batch elements
(embed GEMM -> GRU over 16 context windows -> bilinear scoring),
parameters replicated, no collectives.

Input staging (host, inside kernel(), like the W_embed pre-chunk/cast):
X ships to device DRAM transposed AND pass-packed in bf16.  For each of
3 column passes p (a 1024-column block-pair of the 3072 per-core
windows), XTp[q, j*1024+c] = X^T[k=j*128+q, row=p*1024+c]: partition q
holds k-chunk j's row contiguously, so a 4-chunk [128, 4096] tile is ONE
1-MB DMA with 8-KB-contiguous per-partition segments -- near-line-rate
HBM streaming (17 DMAs per pass, alternating gpsimd SWDGE / sync HWDGE
issue queues).

Device kernel = streaming GEMM at the bf16 HBM roofline: per chunk two
accumulating matmuls (W chunk stationary) into the pass's two PSUM
banks; bias-evac to ET[100, 3072] per pass.  No on-chip transposes.

The serial GRU chain runs on DVE+ACT only (gpsimd does nothing but DMA
issue, so the chain never queues behind buffer-full DMA waits -- in v6
that stretched the 16 steps to 150us and serialized them after the
stream).  GRU hides in pass 1, bilinear A+products in pass 2; the tail
is just the nb6-9 products and the float32r ones-matmul reduction.
"""

import numpy as np

import concourse.bacc as bacc
import concourse.mybir as mybir
import concourse.tile as tile
from concourse.bass_utils import run_bass_kernel_spmd

N_CORES = 8
BC = 16          # batch per core
NE = 16          # context windows (gru seq len)
NB = 10          # negative samples
CT = 8400        # flattened window (21*400)
E = 100          # embed dim == gru hidden
ROWS = BC * NE * (2 + NB)   # 3072 rows per core
NCHUNK = 66                 # 8448 / 128 k-chunks (last 48 rows zero-pad)
CTP = NCHUNK * 128          # 8448
NT = 17                     # 4-chunk tiles per pass (last tile: 2 chunks)

F32 = mybir.dt.float32
BF16 = mybir.dt.bfloat16


def _emit(nc, tc, ctx):
    XTs = [nc.dram_tensor(f"XT{p}", [128, NCHUNK * 1024], BF16,
                          kind="ExternalInput").ap() for p in range(3)]
    WembA = nc.dram_tensor("WembA", [128, 8 * E], BF16,
                           kind="ExternalInput").ap()
    WembB = nc.dram_tensor("WembB", [128, (NCHUNK - 8) * E], BF16,
                           kind="ExternalInput").ap()
    bemb = nc.dram_tensor("bemb", [E, 1], F32, kind="ExternalInput").ap()
    WihT = nc.dram_tensor("WihT", [E, 300], F32, kind="ExternalInput").ap()
    WhhT = nc.dram_tensor("WhhT", [E, 300], F32, kind="ExternalInput").ap()
    bias4 = nc.dram_tensor("bias4", [E, 4], F32, kind="ExternalInput").ap()
    Wbil = nc.dram_tensor("Wbil", [E, NE * E], F32, kind="ExternalInput").ap()
    ones = nc.dram_tensor("ones", [E, 1], mybir.dt.float32r,
                          kind="ExternalInput").ap()
    out_d = nc.dram_tensor("out", [1, NE * BC * (NB + 1)], F32,
                           kind="ExternalOutput").ap()

    P = ctx.enter_context  # pools

    const = P(tc.tile_pool(name="const", bufs=1))
    xtp = P(tc.tile_pool(name="xt", bufs=8))
    psE = P(tc.tile_pool(name="psE", bufs=2, space="PSUM"))
    psS = P(tc.tile_pool(name="psS", bufs=1, space="PSUM"))
    small = P(tc.tile_pool(name="small", bufs=2))

    # ---- persistent SBUF ----
    # consts on the fast sync/gpsimd queues: the tile scheduler hoists
    # the first GRU gh matmuls to the front of the in-order PE queue,
    # and a late const (the scalar ring crawls to ~34us behind the X
    # torrent) would stall the whole embed stream behind them
    # W at the HEAD of the gpsimd queue: its 13.2-KB descriptors win
    # the per-packet round-robin and it completes (~5us) before X tile 0
    # lands -- small-descriptor consts on sync get starved to ~40 GB/s
    # by the X torrent, which is fine for everything EXCEPT W
    # critical path to the first matmul: W head (160 KB) + X tile 0,
    # both at the gpsimd head; the W tail rides the sync ring (11.6-KB
    # descriptors win the per-packet RR) and lands before chunk 8 needs it
    W_a = const.tile([128, 8 * E], BF16)
    nc.gpsimd.dma_start(W_a[:], WembA[:])
    xt00 = xtp.tile([128, 4096], BF16, name="xt")
    nc.gpsimd.dma_start(xt00[:], XTs[0][:, 0:4096])
    WhhT_sb = const.tile([E, 300], F32)
    nc.sync.dma_start(WhhT_sb[:], WhhT[:])
    W_b = const.tile([128, (NCHUNK - 8) * E], BF16)
    nc.sync.dma_start(W_b[:], WembB[:])
    WihT_sb = const.tile([E, 300], F32)
    nc.sync.dma_start(WihT_sb[:], WihT[:])
    bemb_sb = const.tile([E, 1], F32)
    nc.sync.dma_start(bemb_sb[:], bemb[:])
    bias4_sb = const.tile([E, 4], F32)
    nc.sync.dma_start(bias4_sb[:], bias4[:])
    Wbil_sb = const.tile([E, NE * E], F32)
    nc.sync.dma_start(Wbil_sb[:], Wbil[:])
    ones_sb = const.tile([E, 1], mybir.dt.float32r)
    nc.sync.dma_start(ones_sb[:], ones[:])

    ET = const.tile([E, ROWS], F32)                # all embeddings, transposed
    gi_sb = const.tile([E, NE * 3 * BC], F32)      # preacts, [s][r|z|n] blocks
    h = const.tile([E, BC], F32)                   # GRU hidden state (h^T)
    tmp_all = const.tile([E, NE * BC * (NB + 1)], mybir.dt.float32r)
    out_sb = const.tile([1, NE * BC * (NB + 1)], F32)

    A_sb = const.tile([E, NE * BC], F32)           # bilinear A, persisted
    gi_v = gi_sb.rearrange("e (s g b) -> e s g b", s=NE, g=3)
    tmp_v = tmp_all.rearrange("e (s b p) -> e s b p", s=NE, b=BC)
    Eb_v = ET[:, 512:ROWS].rearrange("e (nb s b) -> e nb s b", nb=NB, s=NE)

    def bil_part(s0, s1):
        # A_s = W_bil[s].T @ h^T plus the Ep and nb0-5 score products
        # (blocks 0-3, all evacuated by end of pass 1) -- spread over
        # pass 2; the nb6-9 products run as the tail.
        for s in range(s0, s1):
            Ap = psS.tile([E, BC], F32, tag="bilA", name="Ap", bufs=2)
            nc.tensor.matmul(Ap[:, :], Wbil_sb[:, s * E:(s + 1) * E], h[:],
                             start=True, stop=True)
            nc.scalar.copy(A_sb[:, s * BC:(s + 1) * BC], Ap[:])
            nc.vector.tensor_mul(tmp_v[:, s, :, 0],
                                 ET[:, NE * BC + s * BC: NE * BC + (s + 1) * BC],
                                 Ap[:])
            nc.vector.tensor_mul(
                tmp_v[:, s, :, 1:7].rearrange("e b p -> e p b"),
                Eb_v[:, 0:6, s, :],
                Ap[:].unsqueeze(1).broadcast_to([E, 6, BC]))

    def gru_init():
        nc.vector.memset(h[:], 0.0)
        for g in range(3):
            gp = psS.tile([E, NE * BC], F32, tag="sp0", name="gp")
            nc.tensor.matmul(gp[:, :], WihT_sb[:, g * E:(g + 1) * E],
                             ET[:, 0:NE * BC], start=True, stop=True)
            nc.scalar.add(gi_v[:, :, g, :],
                          gp.rearrange("e (s b) -> e s b", s=NE),
                          bias4_sb[:, g:g + 1])

    def gru_step(s):
        # serial chain on DVE (elementwise) + ACT (sigmoid/tanh) only;
        # gpsimd stays free for DMA issue so the chain never stalls
        # behind buffer-full DMA waits
        c0 = s * 3 * BC
        gh = psS.tile([E, 3 * BC], F32, tag="sp1", name="gh")
        for g in range(3):
            nc.tensor.matmul(gh[:, g * BC:(g + 1) * BC],
                             WhhT_sb[:, g * E:(g + 1) * E], h[:],
                             start=True, stop=True)
        rzt = small.tile([E, 2 * BC], F32, tag="rzt", name="rzt")
        nc.vector.tensor_add(rzt[:], gh[:, 0:2 * BC], gi_sb[:, c0:c0 + 2 * BC])
        rz = small.tile([E, 2 * BC], F32, tag="rz", name="rz")
        nc.scalar.activation(rz[:], rzt[:],
                             mybir.ActivationFunctionType.Sigmoid)
        hn = small.tile([E, BC], F32, tag="hn", name="hn")
        nc.vector.tensor_scalar_add(hn[:], gh[:, 2 * BC:3 * BC],
                                    bias4_sb[:, 3:4])  # gh_n + b_hn
        t1 = small.tile([E, BC], F32, tag="t1", name="t1")
        nc.vector.tensor_mul(t1[:], rz[:, 0:BC], hn[:])
        t2 = small.tile([E, BC], F32, tag="t2", name="t2")
        nc.vector.tensor_add(t2[:], t1[:], gi_sb[:, c0 + 2 * BC:c0 + 3 * BC])
        n = small.tile([E, BC], F32, tag="n", name="n")
        nc.scalar.activation(n[:], t2[:], mybir.ActivationFunctionType.Tanh)
        d = small.tile([E, BC], F32, tag="d", name="d")
        nc.vector.tensor_sub(d[:], h[:], n[:])
        zd = small.tile([E, BC], F32, tag="zd", name="zd")
        nc.vector.tensor_mul(zd[:], rz[:, BC:2 * BC], d[:])
        nc.vector.tensor_add(h[:], n[:], zd[:])    # h = n + z*(h-n)

    def Wc(j):
        return (W_a[:, j * E:(j + 1) * E] if j < 8
                else W_b[:, (j - 8) * E:(j - 7) * E])

    # prewarm the ACT sigmoid/tanh tables off the GRU critical path
    warm = small.tile([E, 1], F32, tag="warm", name="warm")
    nc.scalar.activation(warm[:], bias4_sb[:, 0:1],
                         mybir.ActivationFunctionType.Sigmoid)
    nc.scalar.activation(warm[:], bias4_sb[:, 0:1],
                         mybir.ActivationFunctionType.Tanh)

    # ---- embed: 3 passes over k, each a 1024-column block-pair ----
    for p in range(3):
        et0 = psE.tile([E, 512], F32, tag="et0", name="et0")
        et1 = psE.tile([E, 512], F32, tag="et1", name="et1")
        c0 = p * 1024
        for jt in range(NT):
            j0 = jt * 4
            jn = min(4, NCHUNK - j0)
            # interleaved serial work: GRU in pass 1, bilinear in pass 2
            if p == 1 and jt < NE:
                gru_step(jt)
            elif p == 2 and jt % 4 == 0 and jt < 16:
                q = jt // 4
                bil_part(4 * q, 4 * q + 4)
            if p == 0 and jt == 0:
                xt = xt00                      # prefetched at gpsimd head
            else:
                xt = xtp.tile([128, 4096], BF16, name="xt")
                # 2:1 toward SWDGE -- the sync HWDGE ring has the lower
                # per-ring ceiling, so give it a third of the tiles
                eng = nc.gpsimd if jt % 3 != 2 else nc.sync
                eng.dma_start(xt[:, 0:jn * 1024],
                              XTs[p][:, j0 * 1024:(j0 + jn) * 1024])
            for u in range(jn):
                j = j0 + u
                nc.tensor.matmul(et0[:, :], Wc(j),
                                 xt[:, u * 1024:u * 1024 + 512],
                                 start=(j == 0), stop=(j == NCHUNK - 1),
                                 skip_group_check=True)
                nc.tensor.matmul(et1[:, :], Wc(j),
                                 xt[:, u * 1024 + 512:(u + 1) * 1024],
                                 start=(j == 0), stop=(j == NCHUNK - 1),
                                 skip_group_check=True)
        nc.scalar.add(ET[:, c0:c0 + 512], et0[:, :], bemb_sb[:, 0:1])
        nc.scalar.add(ET[:, c0 + 512:c0 + 1024], et1[:, :], bemb_sb[:, 0:1])
        # gi preacts as soon as block 0 (Ec) is done
        if p == 0:
            gru_init()

    # ---- tail: nb6-9 products (blocks 4-5) with the float32r
    # ones-matmul reduction interleaved as soon as each 512-col range of
    # tmp_all is complete ----
    TOT = NE * BC * (NB + 1)

    def reduce_chunk(c):
        cc = c * 512
        w = min(512, TOT - cc)
        rp = psS.tile([1, 512], F32, tag="bilA", name="rp", bufs=2)
        nc.tensor.matmul(rp[0:1, 0:w], ones_sb[:, 0:1], tmp_all[:, cc:cc + w],
                         start=True, stop=True)
        nc.scalar.copy(out_sb[:, cc:cc + w], rp[0:1, 0:w])

    red_after = {2: 0, 5: 1, 8: 2, 11: 3, 14: 4, 15: 5}
    for s in range(NE):
        nc.vector.tensor_mul(
            tmp_v[:, s, :, 7:NB + 1].rearrange("e b p -> e p b"),
            Eb_v[:, 6:10, s, :],
            A_sb[:, s * BC:(s + 1) * BC].unsqueeze(1).broadcast_to([E, 4, BC]))
        if s in red_after:
            reduce_chunk(red_after[s])
    nc.sync.dma_start(out_d[:], out_sb[:])


def build():
    import contextlib
    nc = bacc.Bacc("TRN2", target_bir_lowering=False, debug=False,
                   enable_asserts=False, num_devices=N_CORES)
    with tile.TileContext(nc) as tc:
        with contextlib.ExitStack() as ctx:
            _emit(nc, tc, ctx)
    nc.compile()
    return nc


_NC = None


def make_in_maps(Xc, Xp, Xb, W_embed, b_embed, W_ih, W_hh, b_ih, b_hh, W_bil):
    import ml_dtypes
    B = Xc.shape[0]
    BF = ml_dtypes.bfloat16
    Xc_b = np.asarray(Xc, np.float32).reshape(B, NE, CT).astype(BF)
    Xp_b = np.asarray(Xp, np.float32).reshape(B, NE, CT).astype(BF)
    Xb_b = np.asarray(Xb, np.float32).reshape(B, NE, NB, CT).astype(BF)

    W_embed = np.ascontiguousarray(W_embed, np.float32)
    W_ch = np.zeros((128, NCHUNK * E), np.float32)
    for j in range(NCHUNK):
        kj = min(128, CT - j * 128)
        W_ch[:kj, j * E:(j + 1) * E] = W_embed[j * 128:j * 128 + kj]
    W_ch = W_ch.astype(BF)
    bemb = np.ascontiguousarray(b_embed, np.float32).reshape(E, 1)
    WihT = np.ascontiguousarray(W_ih.T, np.float32)          # [100, 300]
    WhhT = np.ascontiguousarray(W_hh.T, np.float32)
    bias4 = np.stack([b_ih[0:E] + b_hh[0:E],
                      b_ih[E:2 * E] + b_hh[E:2 * E],
                      b_ih[2 * E:3 * E],
                      b_hh[2 * E:3 * E]], axis=1).astype(np.float32)
    Wbil_r = np.ascontiguousarray(
        np.transpose(W_bil, (1, 0, 2)).reshape(E, NE * E), np.float32)
    ones = np.ones((E, 1), np.float32)

    shared = dict(WembA=np.ascontiguousarray(W_ch[:, :8 * E]),
                  WembB=np.ascontiguousarray(W_ch[:, 8 * E:]),
                  bemb=bemb, WihT=WihT, WhhT=WhhT,
                  bias4=bias4, Wbil=Wbil_r, ones=ones)
    in_maps = []
    for c in range(N_CORES):
        sl = slice(c * BC, (c + 1) * BC)
        # rows in ET column order: Xc (s,b) | Xp (s,b) | Xb (nb,s,b)
        A = np.zeros((ROWS, CTP), BF)
        A[0:256, 0:CT] = Xc_b[sl].transpose(1, 0, 2).reshape(256, CT)
        A[256:512, 0:CT] = Xp_b[sl].transpose(1, 0, 2).reshape(256, CT)
        A[512:, 0:CT] = Xb_b[sl].transpose(2, 1, 0, 3).reshape(2560, CT)
        m = dict(shared)
        for p in range(3):
            m[f"XT{p}"] = np.ascontiguousarray(
                A[p * 1024:(p + 1) * 1024]
                .reshape(1024, NCHUNK, 128)
                .transpose(2, 1, 0)
                .reshape(128, NCHUNK * 1024))
        in_maps.append(m)
    return in_maps


def gather(results):
    outs = []
    for c in range(N_CORES):
        o = results[c]["out"].reshape(NE, BC, NB + 1)       # [s, b, p]
        outs.append(np.transpose(o, (1, 0, 2)))             # [b, s, p]
    return np.concatenate(outs, axis=0).astype(np.float32)  # [128, 16, 11]


def kernel(Xc, Xp, Xb, W_embed, b_embed, W_ih, W_hh, b_ih, b_hh, W_bil):
    global _NC
    if _NC is None:
        _NC = build()
    in_maps = make_in_maps(Xc, Xp, Xb, W_embed, b_embed, W_ih, W_hh,
                           b_ih, b_hh, W_bil)
    res = run_bass_kernel_spmd(_NC, in_maps, core_ids=list(range(N_CORES)))
    return gather(res.results)
